# revision 35
# baseline (speedup 1.0000x reference)
"""Trainium2 Bass kernel: grouped similarity-gating normalization.

Reference computation (per batch b, group g, cpg=64 channels, hw=784):
    means[c]  = mean_hw(x[c, :])
    s[hw]     = sum_c x[c, hw] * means[c]
    t         = (s - mean(s)) * rsqrt(var(s) + eps)
    gate      = sigmoid(t * weight[g] + bias[g])
    out[c,hw] = x[c, hw] * gate[hw]

Sharding: data-parallel over batch B=64 across 8 cores (8 batches/core).

Per-core layout: one SBUF tile [128, 4, 784] per batch holds channels
c = 4*p + j (p = partition, j = free chunk) -> contiguous 1.6MB DMAs, and
group(c) = c//64 = p//16, i.e. each group owns a 16-partition band.

  - channel means via one DVE reduce (innermost axis of [128,4,784])
  - s (contraction over channels) via PE: 4 accumulating matmuls with
    lhsT[p, g] = means[p,j] masked to group bands (constant 0/1 indicator
    m8 times means). An extra N=1 matmul column with rhs=means gives
    mu = mean(s) = sum_c means[c]^2 for free.
  - stats on ScalarE: Square+accum_out -> sum(s^2); gate computed as
    sigmoid(s*a + c) in one activation with per-partition scale/bias APs,
    where a = rstd*weight[g], c = bias[g] - mu*a.
  - gate broadcast to the 128 partitions via PE with the transposed
    indicator (mt), then one DVE multiply (j-broadcast AP), DMA out.
"""

import sys

if "/opt/trn_rl_repo" not in sys.path:
    sys.path.insert(0, "/opt/trn_rl_repo")

from contextlib import ExitStack

import numpy as np

import concourse.bacc as bacc
import concourse.bass as bass
import concourse.tile as tile
from concourse import mybir
from concourse.bass_utils import run_bass_kernel_spmd

B, C, H, W = 64, 512, 28, 28
G = 8
HW = H * W          # 784
NCORES = 8
BLOC = B // NCORES  # 8 batches per core
NP = 128            # SBUF partitions
NJ = C // NP        # 4 channel chunks per partition (c = NJ*p + j)
PBAND = NP // G     # 16 partitions per group
EPS = 1e-5
F32 = mybir.dt.float32
MMCHUNK = 512       # max fp32 moving free dim per matmul

_cache: dict = {}

# implementation choices (bisectable)
OUT_ENGINE = "sync"  # "scalar" or "sync" HWDGE ring for output DMAs
MUL_J3 = "gpsimd"    # "gpsimd" or "vector" for the last gating multiply
REDUCE_MODE = "split"  # "split" (DVE j01 + ACT copy-accum j23) or "dve" (one reduce)
# NOTE: tensor_tensor_reduce (custom DVE ucode op) wedges the device under
# the axon/PJRT runtime (NRT_EXEC_UNIT_UNRECOVERABLE) -- keep "plain".
SQ2_MODE = "plain"   # "ttr" (tensor_tensor_reduce) or "plain" (mul + reduce)
MM_DTYPE = "fp32"    # "fp32" (2-pass, exact) or "fp32r" (1-pass, rounded ~tf32)
N_POOL_MULS = 1      # how many of the 4 gating multiplies run on GpSimd
S_MODE = "hybrid"    # "pe4": 4 PE contraction passes; "hybrid": PE j0/j1 +
                     # z = m2*x2 + m3*x3 on GpSimd/DVE, band-summed on PE


def _emit(tc, nc, xs, m8, wv, bv, ys):
    AF = mybir.ActivationFunctionType
    OP = mybir.AluOpType
    PREF = 3  # input prefetch depth (batches)
    with ExitStack() as ctx:
        consts = ctx.enter_context(tc.tile_pool(name="consts", bufs=1))
        xpool = ctx.enter_context(tc.tile_pool(name="xpool", bufs=BLOC))
        mpool = ctx.enter_context(tc.tile_pool(name="mpool", bufs=3))
        vpool = ctx.enter_context(tc.tile_pool(name="vpool", bufs=4))
        gpool = ctx.enter_context(tc.tile_pool(name="gpool", bufs=3))
        spsum = ctx.enter_context(tc.tile_pool(name="spsum", bufs=2, space="PSUM"))

        # m8 input now carries the [NP, NP] block-banded 0/1 indicator
        # M16[p, q] = (p//PBAND == q//PBAND); wv/bv are 16x-replicated [NP, 1]
        m16_sb = consts.tile([NP, NP], F32)
        nc.sync.dma_start(out=m16_sb[:], in_=m8[:])
        wv_sb = consts.tile([NP, 1], F32)
        nc.sync.dma_start(out=wv_sb[:], in_=wv[:])
        bv_sb = consts.tile([NP, 1], F32)
        nc.sync.dma_start(out=bv_sb[:], in_=bv[:])
        eps_sb = consts.tile([NP, 1], F32)
        nc.vector.memset(eps_sb[:], EPS)

        xts = {}
        state = {}

        def dma_in(b):
            # HW+1 free elems per j: column HW later holds means[:, j] so the
            # matmul's second chunk also accumulates mu = sum(means^2) for free
            xt = xpool.tile([NP, NJ, HW + 1], F32)
            # two chunks so the means reduce can start on the first half
            nc.sync.dma_start(out=xt[:, 0:2, 0:HW], in_=xs[b, :, 0:2, :])
            nc.sync.dma_start(out=xt[:, 2:4, 0:HW], in_=xs[b, :, 2:4, :])
            xts[b] = xt

        def phase1(b):
            # means + masked lhsT (all pre-matmul per-batch prep)
            xt = xts[b]
            means = mpool.tile([NP, NJ], F32, tag="means")
            if REDUCE_MODE == "split":
                sums01 = mpool.tile([NP, 2], F32, tag="sums01")
                nc.vector.reduce_sum(out=sums01[:], in_=xt[:, 0:2, 0:HW], axis=mybir.AxisListType.X)
                nc.vector.tensor_scalar_mul(means[:, 0:2], sums01[:], 1.0 / HW)
                cps = gpool.tile([NP, HW], F32, tag="cps")
                for j in (2, 3):
                    nc.scalar.activation(
                        out=cps[:], in_=xt[:, j, 0:HW], func=AF.Copy,
                        scale=1.0 / HW, accum_out=means[:, j : j + 1],
                    )
            else:
                sums = mpool.tile([NP, NJ], F32, tag="sums")
                nc.vector.reduce_sum(out=sums[:], in_=xt[:, :, 0:HW], axis=mybir.AxisListType.X)
                nc.vector.tensor_scalar_mul(means[:], sums[:], 1.0 / HW)

            # stash means[:, j] in column HW of xt so the second matmul chunk
            # accumulates mu[g] = sum_{c in g} means_c^2 into ps[:, HW]
            for j in range(NJ):
                nc.vector.tensor_copy(xt[:, j, HW : HW + 1], means[:, j : j + 1])

            # lhsT[:, j, q] = means[p, j] masked to the 16-wide band of q, so the
            # matmul emits s replicated onto all 128 PSUM partitions (M=128 is
            # free: PE cost is N-bound)
            npej = 2 if S_MODE == "hybrid" else NJ
            lhsT = mpool.tile([NP, npej, NP], F32, tag="lhsT")
            for j in range(npej):
                nc.vector.tensor_scalar_mul(lhsT[:, j, :], m16_sb[:], means[:, j : j + 1])

            if S_MODE == "hybrid":
                # z = m2*x2 + m3*x3 (incl. the means column) off the PE:
                # GpSimd does the single-input scale, DVE the fused mul-add
                zb = mpool.tile([NP, HW + 1], F32, tag="zb")
                nc.gpsimd.tensor_scalar_mul(zb[:], xt[:, 2, :], means[:, 2:3])
                nc.vector.scalar_tensor_tensor(
                    out=zb[:], in0=xt[:, 3, :], scalar=means[:, 3:4], in1=zb[:],
                    op0=OP.mult, op1=OP.add,
                )
                state[b] = (lhsT, zb)
            else:
                state[b] = (lhsT, None)

        def phase2(b):
            # s (replicated per 16-band) in cols 0:HW; replicated mu in col HW
            xt = xts[b]
            lhsT, zb = state[b]
            npej = 2 if S_MODE == "hybrid" else NJ
            ps = spsum.tile([NP, HW + 1], F32)
            for c0 in range(0, HW + 1, MMCHUNK):
                c1 = min(c0 + MMCHUNK, HW + 1)
                passes = [(lhsT[:, j, :], xt[:, j, c0:c1]) for j in range(npej)]
                if zb is not None:
                    passes.append((m16_sb[:], zb[:, c0:c1]))
                for k, (lw, rw) in enumerate(passes):
                    st = dict(start=(k == 0), stop=(k == len(passes) - 1))
                    if MM_DTYPE == "fp32r":
                        lw = lw.bitcast(mybir.dt.float32r)
                        rw = rw.bitcast(mybir.dt.float32r)
                    nc.tensor.matmul(ps[:, c0:c1], lw, rw, **st)
            state[b] = ps

        def phase3(b):
            # stats + gate (everything already replicated on 128 partitions)
            ps = state[b]
            nmu = vpool.tile([NP, 1], F32, tag="nmu")
            nc.vector.tensor_scalar_mul(nmu[:], ps[:, HW : HW + 1], -1.0)
            sq = gpool.tile([NP, HW], F32, tag="sq")
            hwvar = vpool.tile([NP, 1], F32, tag="hwvar")
            nc.scalar.activation(
                out=sq[:], in_=ps[:, 0:HW], func=AF.Square, bias=nmu[:], accum_out=hwvar[:]
            )
            std = vpool.tile([NP, 1], F32, tag="std")
            nc.scalar.activation(
                out=std[:], in_=hwvar[:], func=AF.Sqrt, scale=1.0 / HW, bias=eps_sb[:]
            )
            rstd = vpool.tile([NP, 1], F32, tag="rstd")
            nc.vector.reciprocal(rstd[:], std[:])
            a_t = vpool.tile([NP, 1], F32, tag="a_t")
            nc.vector.tensor_mul(a_t[:], rstd[:], wv_sb[:])
            c_t = vpool.tile([NP, 1], F32, tag="c_t")
            nc.vector.scalar_tensor_tensor(
                out=c_t[:], in0=nmu[:], scalar=a_t[:], in1=bv_sb[:],
                op0=OP.mult, op1=OP.add,
            )
            gate = gpool.tile([NP, HW], F32, tag="gate")
            nc.scalar.activation(
                out=gate[:], in_=ps[:, 0:HW], func=AF.Sigmoid, bias=c_t[:], scale=a_t[:]
            )
            state[b] = gate[:]

        def phase4(b):
            # gating multiply (in place) + store
            xt = xts.pop(b)
            bg_ap = state.pop(b)
            npool = N_POOL_MULS if MUL_J3 == "gpsimd" else 0
            for j in range(NJ):
                eng = nc.gpsimd if j >= NJ - npool else nc.vector
                eng.tensor_mul(xt[:, j, 0:HW], xt[:, j, 0:HW], bg_ap)
            if OUT_ENGINE == "scalar":
                nc.scalar.dma_start(out=ys[b], in_=xt[:, :, 0:HW])
            else:
                nc.sync.dma_start(out=ys[b], in_=xt[:, :, 0:HW])
            if b + PREF < BLOC:
                dma_in(b + PREF)

        # software-pipelined emission: each engine's stream sees work in
        # data-readiness order, so in-order engines never head-of-line block
        for b in range(PREF):
            dma_in(b)
        phase1(0)
        phase2(0)
        for b in range(BLOC):
            if b + 1 < BLOC:
                phase1(b + 1)
            phase3(b)
            if b + 1 < BLOC:
                phase2(b + 1)
            phase4(b)


def _build_nc():
    nc = bacc.Bacc("TRN2", debug=False)
    xs = nc.dram_tensor("xs", [BLOC, NP, NJ, HW], F32, kind="ExternalInput")
    m8 = nc.dram_tensor("m8", [NP, NP], F32, kind="ExternalInput")
    wv = nc.dram_tensor("wv", [NP, 1], F32, kind="ExternalInput")
    bv = nc.dram_tensor("bv", [NP, 1], F32, kind="ExternalInput")
    ys = nc.dram_tensor("ys", [BLOC, NP, NJ, HW], F32, kind="ExternalOutput")
    with tile.TileContext(nc) as tc:
        _emit(tc, nc, xs, m8, wv, bv, ys)
    nc.compile()
    return nc


def get_nc():
    if "nc" not in _cache:
        _cache["nc"] = _build_nc()
    return _cache["nc"]


def make_in_maps(x, weight, bias):
    x = np.ascontiguousarray(np.asarray(x, dtype=np.float32))
    weight = np.asarray(weight, dtype=np.float32).reshape(G)
    bias = np.asarray(bias, dtype=np.float32).reshape(G)
    # [core, b, p, j, hw] with c = NJ*p + j
    xs = x.reshape(NCORES, BLOC, NP, NJ, HW)
    band = np.arange(NP) // PBAND
    m8 = (band[:, None] == band[None, :]).astype(np.float32)  # [NP, NP] indicator
    wv = np.ascontiguousarray(np.repeat(weight, PBAND)[:, None])
    bv = np.ascontiguousarray(np.repeat(bias, PBAND)[:, None])
    return [
        {"xs": np.ascontiguousarray(xs[i]), "m8": m8, "wv": wv, "bv": bv}
        for i in range(NCORES)
    ]


def run(x, weight, bias, trace=False, **spmd_kwargs):
    nc = get_nc()
    in_maps = make_in_maps(x, weight, bias)
    res = run_bass_kernel_spmd(
        nc, in_maps, core_ids=list(range(NCORES)), trace=trace, **spmd_kwargs
    )
    out = np.stack([res.results[i]["ys"] for i in range(NCORES)])
    return out.reshape(B, C, H, W), res


def kernel(x, weight, bias, groups=G, **_ignored):
    assert int(groups) == G
    out, _ = run(x, weight, bias, trace=False)
    return out


# revision 37
# speedup vs baseline: 1.5375x; 1.5375x over previous
"""Trainium2 Bass kernel: grouped similarity-gating normalization.

Reference computation (per batch b, group g, cpg=64 channels, hw=784):
    means[c]  = mean_hw(x[c, :])
    s[hw]     = sum_c x[c, hw] * means[c]
    t         = (s - mean(s)) * rsqrt(var(s) + eps)
    gate      = sigmoid(t * weight[g] + bias[g])
    out[c,hw] = x[c, hw] * gate[hw]

Sharding: data-parallel over batch B=64 across 8 cores (8 batches/core).

Per-core layout: one SBUF tile [128, 4, 784] per batch holds channels
c = 4*p + j (p = partition, j = free chunk) -> contiguous 1.6MB DMAs, and
group(c) = c//64 = p//16, i.e. each group owns a 16-partition band.

  - channel means via one DVE reduce (innermost axis of [128,4,784])
  - s (contraction over channels) via PE: 4 accumulating matmuls with
    lhsT[p, g] = means[p,j] masked to group bands (constant 0/1 indicator
    m8 times means). An extra N=1 matmul column with rhs=means gives
    mu = mean(s) = sum_c means[c]^2 for free.
  - stats on ScalarE: Square+accum_out -> sum(s^2); gate computed as
    sigmoid(s*a + c) in one activation with per-partition scale/bias APs,
    where a = rstd*weight[g], c = bias[g] - mu*a.
  - gate broadcast to the 128 partitions via PE with the transposed
    indicator (mt), then one DVE multiply (j-broadcast AP), DMA out.
"""

import sys

if "/opt/trn_rl_repo" not in sys.path:
    sys.path.insert(0, "/opt/trn_rl_repo")

from contextlib import ExitStack

import numpy as np

import concourse.bacc as bacc
import concourse.bass as bass
import concourse.tile as tile
from concourse import mybir
from concourse.bass_utils import run_bass_kernel_spmd

B, C, H, W = 64, 512, 28, 28
G = 8
HW = H * W          # 784
NCORES = 8
BLOC = B // NCORES  # 8 batches per core
NP = 128            # SBUF partitions
NJ = C // NP        # 4 channel chunks per partition (c = NJ*p + j)
PBAND = NP // G     # 16 partitions per group
EPS = 1e-5
F32 = mybir.dt.float32
MMCHUNK = 512       # max fp32 moving free dim per matmul

_cache: dict = {}

# implementation choices (bisectable)
OUT_ENGINE = "sync"  # "scalar" or "sync" HWDGE ring for output DMAs
MUL_J3 = "gpsimd"    # "gpsimd" or "vector" for the last gating multiply
REDUCE_MODE = "split"  # "split" (DVE j01 + ACT copy-accum j23) or "dve" (one reduce)
# NOTE: tensor_tensor_reduce (custom DVE ucode op) wedges the device under
# the axon/PJRT runtime (NRT_EXEC_UNIT_UNRECOVERABLE) -- keep "plain".
SQ2_MODE = "plain"   # "ttr" (tensor_tensor_reduce) or "plain" (mul + reduce)
MM_DTYPE = "fp32"    # "fp32" (2-pass, exact) or "fp32r" (1-pass, rounded ~tf32)
N_POOL_MULS = 2      # how many of the 4 gating multiplies run on GpSimd
S_MODE = "hybrid"    # "pe4": 4 PE contraction passes; "hybrid": PE j0/j1 +
                     # z = m2*x2 + m3*x3 on GpSimd/DVE, band-summed on PE


def _emit(tc, nc, xs, m8, wv, bv, ys):
    AF = mybir.ActivationFunctionType
    OP = mybir.AluOpType
    PREF = 3  # input prefetch depth (batches)
    with ExitStack() as ctx:
        consts = ctx.enter_context(tc.tile_pool(name="consts", bufs=1))
        xpool = ctx.enter_context(tc.tile_pool(name="xpool", bufs=BLOC))
        mpool = ctx.enter_context(tc.tile_pool(name="mpool", bufs=3))
        vpool = ctx.enter_context(tc.tile_pool(name="vpool", bufs=4))
        gpool = ctx.enter_context(tc.tile_pool(name="gpool", bufs=3))
        spsum = ctx.enter_context(tc.tile_pool(name="spsum", bufs=2, space="PSUM"))

        # m8 input now carries the [NP, NP] block-banded 0/1 indicator
        # M16[p, q] = (p//PBAND == q//PBAND); wv/bv are 16x-replicated [NP, 1]
        m16_sb = consts.tile([NP, NP], F32)
        nc.sync.dma_start(out=m16_sb[:], in_=m8[:])
        wv_sb = consts.tile([NP, 1], F32)
        nc.sync.dma_start(out=wv_sb[:], in_=wv[:])
        bv_sb = consts.tile([NP, 1], F32)
        nc.sync.dma_start(out=bv_sb[:], in_=bv[:])
        eps_sb = consts.tile([NP, 1], F32)
        nc.vector.memset(eps_sb[:], EPS)

        xts = {}
        state = {}

        def dma_in(b):
            # HW+1 free elems per j: column HW later holds means[:, j] so the
            # matmul's second chunk also accumulates mu = sum(means^2) for free
            xt = xpool.tile([NP, NJ, HW + 1], F32)
            # two chunks so the means reduce can start on the first half
            nc.sync.dma_start(out=xt[:, 0:2, 0:HW], in_=xs[b, :, 0:2, :])
            nc.sync.dma_start(out=xt[:, 2:4, 0:HW], in_=xs[b, :, 2:4, :])
            xts[b] = xt

        def phase1(b):
            # means + masked lhsT (all pre-matmul per-batch prep)
            xt = xts[b]
            means = mpool.tile([NP, NJ], F32, tag="means")
            if REDUCE_MODE == "split":
                sums01 = mpool.tile([NP, 2], F32, tag="sums01")
                nc.vector.reduce_sum(out=sums01[:], in_=xt[:, 0:2, 0:HW], axis=mybir.AxisListType.X)
                nc.vector.tensor_scalar_mul(means[:, 0:2], sums01[:], 1.0 / HW)
                cps = gpool.tile([NP, HW], F32, tag="cps")
                for j in (2, 3):
                    nc.scalar.activation(
                        out=cps[:], in_=xt[:, j, 0:HW], func=AF.Copy,
                        scale=1.0 / HW, accum_out=means[:, j : j + 1],
                    )
            else:
                sums = mpool.tile([NP, NJ], F32, tag="sums")
                nc.vector.reduce_sum(out=sums[:], in_=xt[:, :, 0:HW], axis=mybir.AxisListType.X)
                nc.vector.tensor_scalar_mul(means[:], sums[:], 1.0 / HW)

            # stash means[:, j] in column HW of xt so the second matmul chunk
            # accumulates mu[g] = sum_{c in g} means_c^2 into ps[:, HW]
            for j in range(NJ):
                nc.vector.tensor_copy(xt[:, j, HW : HW + 1], means[:, j : j + 1])

            # lhsT[:, j, q] = means[p, j] masked to the 16-wide band of q, so the
            # matmul emits s replicated onto all 128 PSUM partitions (M=128 is
            # free: PE cost is N-bound)
            npej = 2 if S_MODE == "hybrid" else NJ
            lhsT = mpool.tile([NP, npej, NP], F32, tag="lhsT")
            for j in range(npej):
                nc.vector.tensor_scalar_mul(lhsT[:, j, :], m16_sb[:], means[:, j : j + 1])

            if S_MODE == "hybrid":
                # z = m2*x2 + m3*x3 (incl. the means column) off the PE
                # (keep off GpSimd: its TensorScalar ucode measures ~11us/op)
                zb = mpool.tile([NP, HW + 1], F32, tag="zb")
                nc.vector.tensor_scalar_mul(zb[:], xt[:, 2, :], means[:, 2:3])
                nc.vector.scalar_tensor_tensor(
                    out=zb[:], in0=xt[:, 3, :], scalar=means[:, 3:4], in1=zb[:],
                    op0=OP.mult, op1=OP.add,
                )
                state[b] = (lhsT, zb)
            else:
                state[b] = (lhsT, None)

        def phase2(b):
            # s (replicated per 16-band) in cols 0:HW; replicated mu in col HW
            xt = xts[b]
            lhsT, zb = state[b]
            npej = 2 if S_MODE == "hybrid" else NJ
            ps = spsum.tile([NP, HW + 1], F32)
            for c0 in range(0, HW + 1, MMCHUNK):
                c1 = min(c0 + MMCHUNK, HW + 1)
                passes = [(lhsT[:, j, :], xt[:, j, c0:c1]) for j in range(npej)]
                if zb is not None:
                    passes.append((m16_sb[:], zb[:, c0:c1]))
                for k, (lw, rw) in enumerate(passes):
                    st = dict(start=(k == 0), stop=(k == len(passes) - 1))
                    if MM_DTYPE == "fp32r":
                        lw = lw.bitcast(mybir.dt.float32r)
                        rw = rw.bitcast(mybir.dt.float32r)
                    nc.tensor.matmul(ps[:, c0:c1], lw, rw, **st)
            state[b] = ps

        def phase3(b):
            # stats + gate (everything already replicated on 128 partitions)
            ps = state[b]
            nmu = vpool.tile([NP, 1], F32, tag="nmu")
            nc.vector.tensor_scalar_mul(nmu[:], ps[:, HW : HW + 1], -1.0)
            sq = gpool.tile([NP, HW], F32, tag="sq")
            hwvar = vpool.tile([NP, 1], F32, tag="hwvar")
            nc.scalar.activation(
                out=sq[:], in_=ps[:, 0:HW], func=AF.Square, bias=nmu[:], accum_out=hwvar[:]
            )
            std = vpool.tile([NP, 1], F32, tag="std")
            nc.scalar.activation(
                out=std[:], in_=hwvar[:], func=AF.Sqrt, scale=1.0 / HW, bias=eps_sb[:]
            )
            rstd = vpool.tile([NP, 1], F32, tag="rstd")
            nc.vector.reciprocal(rstd[:], std[:])
            a_t = vpool.tile([NP, 1], F32, tag="a_t")
            nc.vector.tensor_mul(a_t[:], rstd[:], wv_sb[:])
            c_t = vpool.tile([NP, 1], F32, tag="c_t")
            nc.vector.scalar_tensor_tensor(
                out=c_t[:], in0=nmu[:], scalar=a_t[:], in1=bv_sb[:],
                op0=OP.mult, op1=OP.add,
            )
            gate = gpool.tile([NP, HW], F32, tag="gate")
            nc.scalar.activation(
                out=gate[:], in_=ps[:, 0:HW], func=AF.Sigmoid, bias=c_t[:], scale=a_t[:]
            )
            state[b] = gate[:]

        def phase4(b):
            # gating multiply (in place) + store
            xt = xts.pop(b)
            bg_ap = state.pop(b)
            npool = N_POOL_MULS if MUL_J3 == "gpsimd" else 0
            for j in range(NJ):
                eng = nc.gpsimd if j >= NJ - npool else nc.vector
                eng.tensor_mul(xt[:, j, 0:HW], xt[:, j, 0:HW], bg_ap)
            if OUT_ENGINE == "scalar":
                nc.scalar.dma_start(out=ys[b], in_=xt[:, :, 0:HW])
            else:
                nc.sync.dma_start(out=ys[b], in_=xt[:, :, 0:HW])
            if b + PREF < BLOC:
                dma_in(b + PREF)

        # software-pipelined emission: each engine's stream sees work in
        # data-readiness order, so in-order engines never head-of-line block
        for b in range(PREF):
            dma_in(b)
        phase1(0)
        phase2(0)
        for b in range(BLOC):
            if b + 1 < BLOC:
                phase1(b + 1)
            phase3(b)
            if b + 1 < BLOC:
                phase2(b + 1)
            phase4(b)


def _build_nc():
    nc = bacc.Bacc("TRN2", debug=False)
    xs = nc.dram_tensor("xs", [BLOC, NP, NJ, HW], F32, kind="ExternalInput")
    m8 = nc.dram_tensor("m8", [NP, NP], F32, kind="ExternalInput")
    wv = nc.dram_tensor("wv", [NP, 1], F32, kind="ExternalInput")
    bv = nc.dram_tensor("bv", [NP, 1], F32, kind="ExternalInput")
    ys = nc.dram_tensor("ys", [BLOC, NP, NJ, HW], F32, kind="ExternalOutput")
    with tile.TileContext(nc) as tc:
        _emit(tc, nc, xs, m8, wv, bv, ys)
    nc.compile()
    return nc


def get_nc():
    if "nc" not in _cache:
        _cache["nc"] = _build_nc()
    return _cache["nc"]


def make_in_maps(x, weight, bias):
    x = np.ascontiguousarray(np.asarray(x, dtype=np.float32))
    weight = np.asarray(weight, dtype=np.float32).reshape(G)
    bias = np.asarray(bias, dtype=np.float32).reshape(G)
    # [core, b, p, j, hw] with c = NJ*p + j
    xs = x.reshape(NCORES, BLOC, NP, NJ, HW)
    band = np.arange(NP) // PBAND
    m8 = (band[:, None] == band[None, :]).astype(np.float32)  # [NP, NP] indicator
    wv = np.ascontiguousarray(np.repeat(weight, PBAND)[:, None])
    bv = np.ascontiguousarray(np.repeat(bias, PBAND)[:, None])
    return [
        {"xs": np.ascontiguousarray(xs[i]), "m8": m8, "wv": wv, "bv": bv}
        for i in range(NCORES)
    ]


def run(x, weight, bias, trace=False, **spmd_kwargs):
    nc = get_nc()
    in_maps = make_in_maps(x, weight, bias)
    res = run_bass_kernel_spmd(
        nc, in_maps, core_ids=list(range(NCORES)), trace=trace, **spmd_kwargs
    )
    out = np.stack([res.results[i]["ys"] for i in range(NCORES)])
    return out.reshape(B, C, H, W), res


def kernel(x, weight, bias, groups=G, **_ignored):
    assert int(groups) == G
    out, _ = run(x, weight, bias, trace=False)
    return out
